# revision 10
# baseline (speedup 1.0000x reference)
"""Multi-head attention (B=2, S=2048, D=1024, H=16) on 8 NeuronCores.

Sharding: batch x head-group (2 batches x 4 groups of 4 heads). Each core:
  - loads x inputs chunk-wise on the Sync DMA queue (xk first), weights
    batched on the Scalar DMA queue, so the first projection matmul can
    start as soon as xk chunk 0 lands
  - projects Q^T/K^T (kc-outer, trailing the chunk DMAs) and V
  - attention per head-pair: scores via fp16 matmuls, exp on ScalarE
    (fp16 out), attn@V with the per-head row-sums fused in as a ones
    column appended to V (M=65 matmuls) - no separate row-sum matmuls
  - softmax normalization via a broadcast matmul + reciprocal + muls
  - partial output projection y_g^T = Wo[:, g] @ out_g^T, fp16 output
Host: y[b] = sum_g y_g^T.T + bv @ Wo.T + bo.  K-bias drops out of softmax
(per-row constant); Q-bias applied on device; V-bias commutes through the
attention average (rows of attn sum to 1) and is folded host-side.
"""
import numpy as np

B = 2
S = 2048
D = 1024
H = 16
DK = 64
G = 4              # head-groups (cores per batch)
HG = H // G        # heads per group = 4
DH = HG * DK       # group dims = 256
NQB = S // 512     # query blocks
NKC = S // 128     # key chunks
KCD = D // 128     # d_model chunks
VW = 260           # v tile cols per key chunk: 2 pairs x [vA(64)|1|vB(64)|1]

_CACHE = {}


def _build_nc():
    import concourse.tile as tile
    import concourse.bacc as bacc
    from concourse import mybir
    from contextlib import ExitStack

    F32 = mybir.dt.float32
    F16 = mybir.dt.float16
    Exp = mybir.ActivationFunctionType.Exp

    nc = bacc.Bacc("TRN2", target_bir_lowering=False, debug=False)

    xq_d = nc.dram_tensor("xq", [D, S], F16, kind="ExternalInput").ap()
    xk_d = nc.dram_tensor("xk", [D, S], F16, kind="ExternalInput").ap()
    xv_d = nc.dram_tensor("xv", [D, S], F16, kind="ExternalInput").ap()
    wq_d = nc.dram_tensor("wq", [KCD, 128, DH], F16, kind="ExternalInput").ap()
    wk_d = nc.dram_tensor("wk", [KCD, 128, DH], F16, kind="ExternalInput").ap()
    wv_d = nc.dram_tensor("wv", [KCD, 128, DH], F16, kind="ExternalInput").ap()
    wo_d = nc.dram_tensor("wo", [2, 128, D], F16, kind="ExternalInput").ap()
    bq_d = nc.dram_tensor("bq", [128, 2], F32, kind="ExternalInput").ap()
    sel_d = nc.dram_tensor("sel", [128, 64], F16, kind="ExternalInput").ap()
    zr_d = nc.dram_tensor("zr", [128, 1024], F16, kind="ExternalInput").ap()
    ones_d = nc.dram_tensor("ones", [128, 1], F16, kind="ExternalInput").ap()
    y_d = nc.dram_tensor("y", [D, S], F16, kind="ExternalOutput").ap()

    with tile.TileContext(nc) as tc, ExitStack() as ctx:
        sbw = ctx.enter_context(tc.tile_pool(name="sbw", bufs=1))
        sbx = ctx.enter_context(tc.tile_pool(name="sbx", bufs=1))
        sbd = ctx.enter_context(tc.tile_pool(name="sbd", bufs=1))
        sbe = ctx.enter_context(tc.tile_pool(name="sbe", bufs=1))
        sbo = ctx.enter_context(tc.tile_pool(name="sbo", bufs=1))
        ps = ctx.enter_context(tc.tile_pool(name="ps", bufs=1, space="PSUM"))

        # ---- x inputs: xk (first chunk split for early start) + xq on the
        # sync queue; xv rides the scalar queue after the weights ----------
        xk0_t = [sbx.tile([128, 512], F16, name=f"xk0_{j}") for j in range(4)]
        xk_t = [None] + [sbx.tile([128, S], F16, name=f"xk{kc}")
                         for kc in range(1, KCD)]
        xq_t = [sbx.tile([128, S], F16, name=f"xq{kc}") for kc in range(KCD)]
        xv_t = [sbx.tile([128, S], F16, name=f"xv{kc}") for kc in range(KCD)]
        for j in range(4):
            nc.sync.dma_start(xk0_t[j][:], xk_d[0:128, j * 512:(j + 1) * 512])
        for kc in range(1, KCD):
            nc.sync.dma_start(xk_t[kc][:], xk_d[kc * 128:(kc + 1) * 128, :])
        for kc in range(KCD):
            nc.sync.dma_start(xq_t[kc][:], xq_d[kc * 128:(kc + 1) * 128, :])

        def xk_ap(kc, qb):
            if kc == 0:
                return xk0_t[qb][:]
            return xk_t[kc][:, qb * 512:(qb + 1) * 512]

        # ---- weights: batched 3D-AP DMAs on the scalar queue --------------
        wk_t = sbw.tile([128, KCD * DH], F16)
        wq_t = sbw.tile([128, KCD * DH], F16)
        wv_t = sbw.tile([128, KCD * DH], F16)
        wo_t = sbw.tile([128, 2 * D], F16)
        nc.scalar.dma_start(
            wk_t[:].rearrange("p (c f) -> p c f", c=KCD),
            wk_d.transpose([1, 0, 2]))
        nc.scalar.dma_start(
            wq_t[:].rearrange("p (c f) -> p c f", c=KCD),
            wq_d.transpose([1, 0, 2]))
        nc.scalar.dma_start(
            wv_t[:].rearrange("p (c f) -> p c f", c=KCD),
            wv_d.transpose([1, 0, 2]))
        nc.scalar.dma_start(
            wo_t[:].rearrange("p (c f) -> p c f", c=2),
            wo_d.transpose([1, 0, 2]))
        bq_t = sbw.tile([128, 2], F32)
        nc.scalar.dma_start(bq_t[:], bq_d)
        sel_t = sbw.tile([128, 64], F16)
        nc.scalar.dma_start(sel_t[:], sel_d)
        recip = sbw.tile([128, 1024], F16)
        nc.scalar.dma_start(recip[:], zr_d)
        ones_t = sbw.tile([128, 1], F16)
        nc.scalar.dma_start(ones_t[:], ones_d)
        for kc in range(KCD):
            nc.scalar.dma_start(xv_t[kc][:], xv_d[kc * 128:(kc + 1) * 128, :])

        # ---- projection outputs -------------------------------------------
        qt_t = [sbd.tile([128, S], F16, name=f"qt{p}") for p in range(2)]
        kt_t = [sbd.tile([128, S], F16, name=f"kt{p}") for p in range(2)]
        v_tiles = [sbd.tile([128, VW], F16, name=f"v{tb}") for tb in range(NKC)]
        outsc = [sbd.tile([128, S], F16, name=f"outsc{p}") for p in range(2)]

        # row-sum columns of v (cols {64, 129, 194, 259}), written once
        # before the V-proj copies (which skip them). 1/64 keeps the fp16
        # row-sums below overflow; Wo/64 host-side cancels the 64x outsc.
        for tb in range(NKC):
            ov = v_tiles[tb][:].rearrange("p (a b) -> p a b", a=4)[:, :, 64:65]
            nc.gpsimd.memset(ov, 1.0 / 64.0)

        # ---- K projection: kc-outer, 8 accumulators -----------------------
        def k_evac(pb, qb, a):
            with nc.allow_low_precision(reason="fp16 scores"):
                nc.vector.tensor_copy(kt_t[pb][:, qb * 512:(qb + 1) * 512], a)

        def q_evac(pb, qb, a):
            with nc.allow_low_precision(reason="fp16 scores"):
                nc.vector.tensor_scalar_add(qt_t[pb][:, qb * 512:(qb + 1) * 512],
                                            a, bq_t[:, pb:pb + 1])

        kaccs2 = [ps.tile([128, 1024], F32, name=f"ka{i}", tag="sc", bufs=2)
                  for i in range(2)]
        kaccs1 = [ps.tile([128, 512], F32, name=f"kb{i}",
                          tag=("oA" if i < 2 else "oB"), bufs=2)
                  for i in range(4)]

        def kacc(i):  # i = pb * NQB + qb
            if i < 4:
                return kaccs2[i // 2][:, (i % 2) * 512:(i % 2 + 1) * 512]
            return kaccs1[i - 4][:]

        for kc in range(KCD):
            for pb in range(2):
                for qb in range(NQB):
                    nc.tensor.matmul(
                        kacc(pb * NQB + qb),
                        wk_t[:, kc * DH + pb * 128:kc * DH + (pb + 1) * 128],
                        xk_ap(kc, qb),
                        start=(kc == 0), stop=(kc == KCD - 1))
        for pb in range(2):
            for qb in range(NQB):
                k_evac(pb, qb, kacc(pb * NQB + qb))

        # ---- Q projection: kc-outer, 8 accumulators -----------------------
        qaccs2 = [ps.tile([128, 1024], F32, name=f"qa{i}", tag="sc", bufs=2)
                  for i in range(2)]
        qaccs1 = [ps.tile([128, 512], F32, name=f"qb{i}",
                          tag=("oA" if i < 2 else "oB"), bufs=2)
                  for i in range(4)]

        def qacc(i):
            if i < 4:
                return qaccs2[i // 2][:, (i % 2) * 512:(i % 2 + 1) * 512]
            return qaccs1[i - 4][:]

        for kc in range(KCD):
            for pb in range(2):
                for qb in range(NQB):
                    nc.tensor.matmul(
                        qacc(pb * NQB + qb),
                        wq_t[:, kc * DH + pb * 128:kc * DH + (pb + 1) * 128],
                        xq_t[kc][:, qb * 512:(qb + 1) * 512],
                        start=(kc == 0), stop=(kc == KCD - 1))
        for pb in range(2):
            for qb in range(NQB):
                q_evac(pb, qb, qacc(pb * NQB + qb))

        # ---- V projection: tb-outer ---------------------------------------
        for tb in range(NKC):
            acc = ps.tile([128, DH], F32, name="vacc", tag="sc", bufs=2)
            for kc in range(KCD):
                nc.tensor.matmul(
                    acc[:],
                    xv_t[kc][:, tb * 128:(tb + 1) * 128],
                    wv_t[:, kc * DH:(kc + 1) * DH],
                    start=(kc == 0), stop=(kc == KCD - 1))
            src = acc[:].rearrange("p (a c) -> p a c", a=2)
            dst = v_tiles[tb][:].rearrange("p (a c) -> p a c", a=2)
            with nc.allow_low_precision(reason="fp16 attn weights"):
                nc.vector.tensor_copy(dst[:, :, 0:64], src[:, :, 0:64])
                nc.vector.tensor_copy(dst[:, :, 65:129], src[:, :, 64:128])

        # ---- output projection for one query block ------------------------
        def p3_piece(qb, ypb, ysb):
            yacc = ps.tile([128, 512], F32, name="yacc", tag="sc", bufs=2)
            for kc2 in range(2):
                nc.tensor.matmul(
                    yacc[:],
                    wo_t[:, kc2 * D + ypb * 128:kc2 * D + (ypb + 1) * 128],
                    outsc[kc2][:, qb * 512:(qb + 1) * 512],
                    start=(kc2 == 0), stop=(kc2 == 1))
            with nc.allow_low_precision(reason="fp16 y"):
                nc.vector.tensor_copy(ysb[:, ypb * 512:(ypb + 1) * 512], yacc[:])
            if ypb % 2 == 1:
                nc.sync.dma_start(
                    y_d[(ypb - 1) * 128:(ypb + 1) * 128,
                        qb * 512:(qb + 1) * 512].rearrange("(c p) f -> p c f", p=128),
                    ysb[:, (ypb - 1) * 512:(ypb + 1) * 512].rearrange("p (c f) -> p c f", c=2))

        def p3(qb):
            ysb = sbo.tile([128, 8 * 512], F16, name="ysb", tag="ysb", bufs=2)
            for ypb in range(D // 128):
                p3_piece(qb, ypb, ysb)

        # ---- softmax normalization ----------------------------------------
        def norm(pair, qb, outA, outB):
            # scaled row-sums sit at row 64 of outA (head A) and outB (head B)
            with nc.allow_low_precision(reason="fp16 rowsum"):
                nc.vector.tensor_copy(recip[64:65, 0:512], outA[64:65, :])
                nc.vector.tensor_copy(recip[64:65, 512:1024], outB[64:65, :])
            bc_ps = ps.tile([128, 1024], F32, name="bc", tag="sc", bufs=2)
            nc.tensor.matmul(bc_ps[0:64, 0:512], sel_t[:], recip[:, 0:512],
                             start=True, stop=True)
            nc.tensor.matmul(bc_ps[0:64, 512:1024], sel_t[:], recip[:, 512:1024],
                             start=True, stop=True)
            bc_sb = sbo.tile([64, 1024], F32, name="bc_sb", tag="bcastr", bufs=2)
            nc.vector.reciprocal_approx_fast(bc_sb[:], bc_ps[0:64, :])
            with nc.allow_low_precision(reason="fp16 out"):
                nc.vector.tensor_mul(outsc[pair][0:64, qb * 512:(qb + 1) * 512],
                                     outA[0:64, :], bc_sb[:, 0:512])
                bB = sbo.tile([64, 512], F16, name="bB", tag="bB", bufs=2)
                nc.vector.tensor_mul(bB[:], outB[0:64, :], bc_sb[:, 512:1024])
            # partition shift 0:64 -> 64:128 (engines cannot cross partitions)
            nc.gpsimd.dma_start(outsc[pair][64:128, qb * 512:(qb + 1) * 512],
                                bB[:])

        # ---- attention: flat cross-block software pipeline ----------------
        # blocks pair-interleaved so p3 lumps land next to slack; attn@V
        # trails scores/exp by one iteration across block boundaries; norm
        # of block b runs at iter 2 of block b+1; p3 of query block qb is
        # spread one ypb-piece per iteration over iters 8..15 of block
        # 2*qb+2 (outsc for qb complete by then, incl. the shift DMAs)
        blocks = [(pair, qb) for qb in range(NQB) for pair in range(2)]
        outs = {}
        ets = {}

        def attn_v(i):
            bi, kc = divmod(i, NKC)
            pair, qb = blocks[bi]
            outA, outB = outs[bi]
            et = ets.pop(i)
            vt = v_tiles[kc]
            vbase = pair * 130
            # [v|1] blocks: rows 0:64 = dims, row 64 = scaled rowsum
            nc.tensor.matmul(outA[0:65, :],
                             vt[:, vbase:vbase + 65],
                             et[:, 0:512],
                             start=(kc == 0), stop=(kc == NKC - 1))
            nc.tensor.matmul(outB[0:65, :],
                             vt[:, vbase + 65:vbase + 130],
                             et[:, 512:1024],
                             start=(kc == 0), stop=(kc == NKC - 1))

        ysb_cur = None
        for i in range(len(blocks) * NKC):
            bi, kc = divmod(i, NKC)
            pair, qb = blocks[bi]
            ktp, qtp = kt_t[pair], qt_t[pair]
            if kc == 0:
                outs[bi] = (
                    ps.tile([128, 512], F32, name="outA", tag="oA", bufs=2),
                    ps.tile([128, 512], F32, name="outB", tag="oB", bufs=2))
            sc = ps.tile([128, 1024], F32, name="sc", tag="sc", bufs=2)
            nc.tensor.matmul(sc[:, 0:512],
                             ktp[0:64, kc * 128:(kc + 1) * 128],
                             qtp[0:64, qb * 512:(qb + 1) * 512],
                             start=True, stop=True)
            nc.tensor.matmul(sc[:, 512:1024],
                             ktp[64:128, kc * 128:(kc + 1) * 128],
                             qtp[64:128, qb * 512:(qb + 1) * 512],
                             start=True, stop=True)
            et = sbe.tile([128, 1024], F16, name="et", tag="et", bufs=6)
            ets[i] = et
            with nc.allow_low_precision(reason="fp16 attn weights"):
                nc.scalar.activation(et[:], sc[:], Exp, scale=0.125)
            if kc == 2 and bi > 0:
                pb, qbb = blocks[bi - 1]
                norm(pb, qbb, *outs.pop(bi - 1))
            # p3(qX) spread: pieces 0-3 at block 2X+2 kc 8,10,12,14;
            # pieces 4-7 at block 2X+3 kc 0,2,4,6 (norm(1,qX) and its
            # shift DMA complete well before kc 8 of block 2X+2)
            piece = None
            if bi >= 2 and bi % 2 == 0 and kc >= 8 and kc % 2 == 0:
                piece = (blocks[bi - 2][1], (kc - 8) // 2)
            elif bi >= 3 and bi % 2 == 1 and kc <= 6 and kc % 2 == 0:
                piece = (blocks[bi - 3][1], 4 + kc // 2)
            if piece is not None:
                p3qb, ypb = piece
                if ypb == 0:
                    ysb_cur = sbo.tile([128, 8 * 512], F16, name="ysb",
                                       tag="ysb", bufs=2)
                p3_piece(p3qb, ypb, ysb_cur)
            if i > 0:
                attn_v(i - 1)
        attn_v(len(blocks) * NKC - 1)
        pb, qbb = blocks[-1]
        norm(pb, qbb, *outs.pop(len(blocks) - 1))
        p3(NQB - 1)

    nc.compile()
    return nc


def _get_nc():
    if "nc" not in _CACHE:
        _CACHE["nc"] = _build_nc()
    return _CACHE["nc"]


def kernel(q, k, v, Wq, bq, Wk, bk, Wv, bv, Wo, bo, _trace=False, _tmpdir=None):
    from concourse.bass_utils import run_bass_kernel_spmd

    q = np.asarray(q, np.float32)
    k = np.asarray(k, np.float32)
    v = np.asarray(v, np.float32)
    Wq = np.asarray(Wq, np.float32)
    Wk = np.asarray(Wk, np.float32)
    Wv = np.asarray(Wv, np.float32)
    Wo = np.asarray(Wo, np.float32)
    bq = np.asarray(bq, np.float32)
    bk = np.asarray(bk, np.float32)
    bv = np.asarray(bv, np.float32)
    bo = np.asarray(bo, np.float32)

    nc = _get_nc()

    # broadcast-selector for the row-sum reciprocal: bc rows 0:64 get
    # recip row 64 (rsA), rows 64:128 get recip row 63 (rsB fused) or
    # row 96 (rsB via the fallback M=1 matmul)
    sel = np.zeros((128, 64), np.float16)
    sel[64, :] = 1.0
    zr = np.zeros((128, 1024), np.float16)
    ones = np.ones((128, 1), np.float16)

    xT = {}
    for b in range(B):
        xT[("q", b)] = np.ascontiguousarray(q[b].T).astype(np.float16)
        xT[("k", b)] = np.ascontiguousarray(k[b].T).astype(np.float16)
        xT[("v", b)] = np.ascontiguousarray(v[b].T).astype(np.float16)

    # Effective K weights: bk drops out of softmax entirely (adds a
    # per-query-row constant to the scores).  Q bias applied on device.
    in_maps = []
    for c in range(8):
        b, g = c // G, c % G
        gr = slice(g * DH, (g + 1) * DH)
        in_maps.append({
            "xq": xT[("q", b)],
            "xk": xT[("k", b)],
            "xv": xT[("v", b)],
            "wq": np.ascontiguousarray(Wq[gr, :].T).astype(np.float16).reshape(KCD, 128, DH),
            "wk": np.ascontiguousarray(Wk[gr, :].T).astype(np.float16).reshape(KCD, 128, DH),
            "wv": np.ascontiguousarray(Wv[gr, :].T).astype(np.float16).reshape(KCD, 128, DH),
            "wo": np.ascontiguousarray(Wo[:, gr].T / 64.0).astype(np.float16).reshape(2, 128, D),
            "bq": np.ascontiguousarray(bq[gr].reshape(2, 128).T),
            "sel": sel,
            "zr": zr,
            "ones": ones,
        })

    kwargs = {}
    if _trace:
        kwargs = dict(trace=True, tmpdir=_tmpdir)
    res = run_bass_kernel_spmd(nc, in_maps, core_ids=list(range(8)), **kwargs)

    # host reduce: y[b] = sum_g y_g^T.T  (+ bias terms folded host-side)
    bias_row = bv @ Wo.T + bo                     # [D]
    out = np.empty((B, S, D), np.float32)
    for b in range(B):
        acc = np.zeros((S, D), np.float32)
        for g in range(G):
            acc += res.results[b * G + g]["y"].T.astype(np.float32)
        out[b] = acc + bias_row[None, :]
    if _trace:
        out = (out, res)
    return out


# revision 11
# speedup vs baseline: 1.0371x; 1.0371x over previous
"""Multi-head attention (B=2, S=2048, D=1024, H=16) on 8 NeuronCores.

Sharding: batch x head-group (2 batches x 4 groups of 4 heads). Each core:
  - loads x inputs chunk-wise on the Sync DMA queue (xk first), weights
    batched on the Scalar DMA queue, so the first projection matmul can
    start as soon as xk chunk 0 lands
  - projects Q^T/K^T (kc-outer, trailing the chunk DMAs) and V
  - attention per head-pair: scores via fp16 matmuls, exp on ScalarE
    (fp16 out), attn@V with the per-head row-sums fused in as a ones
    column appended to V (M=65 matmuls) - no separate row-sum matmuls
  - softmax normalization via a broadcast matmul + reciprocal + muls
  - partial output projection y_g^T = Wo[:, g] @ out_g^T, fp16 output
Host: y[b] = sum_g y_g^T.T + bv @ Wo.T + bo.  K-bias drops out of softmax
(per-row constant); Q-bias applied on device; V-bias commutes through the
attention average (rows of attn sum to 1) and is folded host-side.
"""
import numpy as np

B = 2
S = 2048
D = 1024
H = 16
DK = 64
G = 4              # head-groups (cores per batch)
HG = H // G        # heads per group = 4
DH = HG * DK       # group dims = 256
NQB = S // 512     # query blocks
NKC = S // 128     # key chunks
KCD = D // 128     # d_model chunks
VW = 260           # v tile cols per key chunk: 2 pairs x [vA(64)|1|vB(64)|1]

_CACHE = {}


def _build_nc():
    import concourse.tile as tile
    import concourse.bacc as bacc
    from concourse import mybir
    from contextlib import ExitStack

    F32 = mybir.dt.float32
    F16 = mybir.dt.float16
    Exp = mybir.ActivationFunctionType.Exp

    nc = bacc.Bacc("TRN2", target_bir_lowering=False, debug=False)

    xq_d = nc.dram_tensor("xq", [D, S], F16, kind="ExternalInput").ap()
    xk_d = nc.dram_tensor("xk", [D, S], F16, kind="ExternalInput").ap()
    xv_d = nc.dram_tensor("xv", [D, S], F16, kind="ExternalInput").ap()
    wq_d = nc.dram_tensor("wq", [KCD, 128, DH], F16, kind="ExternalInput").ap()
    wk_d = nc.dram_tensor("wk", [KCD, 128, DH], F16, kind="ExternalInput").ap()
    wv_d = nc.dram_tensor("wv", [KCD, 128, DH], F16, kind="ExternalInput").ap()
    wo_d = nc.dram_tensor("wo", [2, 128, D], F16, kind="ExternalInput").ap()
    bq_d = nc.dram_tensor("bq", [128, 2], F32, kind="ExternalInput").ap()
    sel_d = nc.dram_tensor("sel", [128, 64], F16, kind="ExternalInput").ap()
    zr_d = nc.dram_tensor("zr", [128, 1024], F16, kind="ExternalInput").ap()
    ones_d = nc.dram_tensor("ones", [128, 1], F16, kind="ExternalInput").ap()
    y_d = nc.dram_tensor("y", [D, S], F16, kind="ExternalOutput").ap()

    with tile.TileContext(nc) as tc, ExitStack() as ctx:
        sbw = ctx.enter_context(tc.tile_pool(name="sbw", bufs=1))
        sbx = ctx.enter_context(tc.tile_pool(name="sbx", bufs=1))
        sbd = ctx.enter_context(tc.tile_pool(name="sbd", bufs=1))
        sbe = ctx.enter_context(tc.tile_pool(name="sbe", bufs=1))
        sbo = ctx.enter_context(tc.tile_pool(name="sbo", bufs=1))
        ps = ctx.enter_context(tc.tile_pool(name="ps", bufs=1, space="PSUM"))

        # ---- x inputs: xk (first chunk split for early start) + xq on the
        # sync queue; xv rides the scalar queue after the weights ----------
        xk0_t = [sbx.tile([128, 512], F16, name=f"xk0_{j}") for j in range(4)]
        xk_t = [None] + [sbx.tile([128, S], F16, name=f"xk{kc}")
                         for kc in range(1, KCD)]
        xq_t = [sbx.tile([128, S], F16, name=f"xq{kc}") for kc in range(KCD)]
        xv_t = [sbx.tile([128, S], F16, name=f"xv{kc}") for kc in range(KCD)]
        for j in range(4):
            nc.sync.dma_start(xk0_t[j][:], xk_d[0:128, j * 512:(j + 1) * 512])
        for kc in range(1, KCD):
            nc.sync.dma_start(xk_t[kc][:], xk_d[kc * 128:(kc + 1) * 128, :])
        for kc in range(KCD):
            nc.sync.dma_start(xq_t[kc][:], xq_d[kc * 128:(kc + 1) * 128, :])
        for kc in range(KCD):
            nc.sync.dma_start(xv_t[kc][:], xv_d[kc * 128:(kc + 1) * 128, :])

        def xk_ap(kc, qb):
            if kc == 0:
                return xk0_t[qb][:]
            return xk_t[kc][:, qb * 512:(qb + 1) * 512]

        # ---- weights: batched 3D-AP DMAs on the scalar queue --------------
        wk_t = sbw.tile([128, KCD * DH], F16)
        wq_t = sbw.tile([128, KCD * DH], F16)
        wv_t = sbw.tile([128, KCD * DH], F16)
        wo_t = sbw.tile([128, 2 * D], F16)
        nc.scalar.dma_start(
            wk_t[:].rearrange("p (c f) -> p c f", c=KCD),
            wk_d.transpose([1, 0, 2]))
        nc.scalar.dma_start(
            wq_t[:].rearrange("p (c f) -> p c f", c=KCD),
            wq_d.transpose([1, 0, 2]))
        nc.scalar.dma_start(
            wv_t[:].rearrange("p (c f) -> p c f", c=KCD),
            wv_d.transpose([1, 0, 2]))
        nc.scalar.dma_start(
            wo_t[:].rearrange("p (c f) -> p c f", c=2),
            wo_d.transpose([1, 0, 2]))
        bq_t = sbw.tile([128, 2], F32)
        nc.scalar.dma_start(bq_t[:], bq_d)
        sel_t = sbw.tile([128, 64], F16)
        nc.scalar.dma_start(sel_t[:], sel_d)
        recip = sbw.tile([128, 1024], F16)
        nc.scalar.dma_start(recip[:], zr_d)
        ones_t = sbw.tile([128, 1], F16)
        nc.scalar.dma_start(ones_t[:], ones_d)

        # ---- projection outputs -------------------------------------------
        qt_t = [sbd.tile([128, S], F16, name=f"qt{p}") for p in range(2)]
        kt_t = [sbd.tile([128, S], F16, name=f"kt{p}") for p in range(2)]
        v_tiles = [sbd.tile([128, VW], F16, name=f"v{tb}") for tb in range(NKC)]
        outsc = [sbd.tile([128, S], F16, name=f"outsc{p}") for p in range(2)]

        # row-sum columns of v (cols {64, 129, 194, 259}), written once
        # before the V-proj copies (which skip them). 1/64 keeps the fp16
        # row-sums below overflow; Wo/64 host-side cancels the 64x outsc.
        for tb in range(NKC):
            ov = v_tiles[tb][:].rearrange("p (a b) -> p a b", a=4)[:, :, 64:65]
            nc.gpsimd.memset(ov, 1.0 / 64.0)

        # ---- K projection: kc-outer, 8 accumulators -----------------------
        def k_evac(pb, qb, a):
            with nc.allow_low_precision(reason="fp16 scores"):
                nc.vector.tensor_copy(kt_t[pb][:, qb * 512:(qb + 1) * 512], a)

        def q_evac(pb, qb, a):
            with nc.allow_low_precision(reason="fp16 scores"):
                nc.vector.tensor_scalar_add(qt_t[pb][:, qb * 512:(qb + 1) * 512],
                                            a, bq_t[:, pb:pb + 1])

        kaccs2 = [ps.tile([128, 1024], F32, name=f"ka{i}", tag="sc", bufs=2)
                  for i in range(2)]
        kaccs1 = [ps.tile([128, 512], F32, name=f"kb{i}",
                          tag=("oA" if i < 2 else "oB"), bufs=2)
                  for i in range(4)]

        def kacc(i):  # i = pb * NQB + qb
            if i < 4:
                return kaccs2[i // 2][:, (i % 2) * 512:(i % 2 + 1) * 512]
            return kaccs1[i - 4][:]

        for kc in range(KCD):
            for pb in range(2):
                for qb in range(NQB):
                    nc.tensor.matmul(
                        kacc(pb * NQB + qb),
                        wk_t[:, kc * DH + pb * 128:kc * DH + (pb + 1) * 128],
                        xk_ap(kc, qb),
                        start=(kc == 0), stop=(kc == KCD - 1))
        for pb in range(2):
            for qb in range(NQB):
                k_evac(pb, qb, kacc(pb * NQB + qb))

        # ---- Q projection: kc-outer, 8 accumulators -----------------------
        qaccs2 = [ps.tile([128, 1024], F32, name=f"qa{i}", tag="sc", bufs=2)
                  for i in range(2)]
        qaccs1 = [ps.tile([128, 512], F32, name=f"qb{i}",
                          tag=("oA" if i < 2 else "oB"), bufs=2)
                  for i in range(4)]

        def qacc(i):
            if i < 4:
                return qaccs2[i // 2][:, (i % 2) * 512:(i % 2 + 1) * 512]
            return qaccs1[i - 4][:]

        for kc in range(KCD):
            for pb in range(2):
                for qb in range(NQB):
                    nc.tensor.matmul(
                        qacc(pb * NQB + qb),
                        wq_t[:, kc * DH + pb * 128:kc * DH + (pb + 1) * 128],
                        xq_t[kc][:, qb * 512:(qb + 1) * 512],
                        start=(kc == 0), stop=(kc == KCD - 1))
        for pb in range(2):
            for qb in range(NQB):
                q_evac(pb, qb, qacc(pb * NQB + qb))

        # ---- V projection: tb-outer ---------------------------------------
        for tb in range(NKC):
            acc = ps.tile([128, DH], F32, name="vacc", tag="sc", bufs=2)
            for kc in range(KCD):
                nc.tensor.matmul(
                    acc[:],
                    xv_t[kc][:, tb * 128:(tb + 1) * 128],
                    wv_t[:, kc * DH:(kc + 1) * DH],
                    start=(kc == 0), stop=(kc == KCD - 1))
            src = acc[:].rearrange("p (a c) -> p a c", a=2)
            dst = v_tiles[tb][:].rearrange("p (a c) -> p a c", a=2)
            with nc.allow_low_precision(reason="fp16 attn weights"):
                nc.vector.tensor_copy(dst[:, :, 0:64], src[:, :, 0:64])
                nc.vector.tensor_copy(dst[:, :, 65:129], src[:, :, 64:128])

        # ---- output projection for one query block ------------------------
        def p3_piece(qb, ypb, ysb):
            yacc = ps.tile([128, 512], F32, name="yacc", tag="sc", bufs=2)
            for kc2 in range(2):
                nc.tensor.matmul(
                    yacc[:],
                    wo_t[:, kc2 * D + ypb * 128:kc2 * D + (ypb + 1) * 128],
                    outsc[kc2][:, qb * 512:(qb + 1) * 512],
                    start=(kc2 == 0), stop=(kc2 == 1))
            with nc.allow_low_precision(reason="fp16 y"):
                nc.vector.tensor_copy(ysb[:, ypb * 512:(ypb + 1) * 512], yacc[:])
            if ypb % 2 == 1:
                nc.sync.dma_start(
                    y_d[(ypb - 1) * 128:(ypb + 1) * 128,
                        qb * 512:(qb + 1) * 512].rearrange("(c p) f -> p c f", p=128),
                    ysb[:, (ypb - 1) * 512:(ypb + 1) * 512].rearrange("p (c f) -> p c f", c=2))

        def p3(qb):
            ysb = sbo.tile([128, 8 * 512], F16, name="ysb", tag="ysb", bufs=2)
            for ypb in range(D // 128):
                p3_piece(qb, ypb, ysb)

        # ---- softmax normalization ----------------------------------------
        def norm(pair, qb, outA, outB):
            # scaled row-sums sit at row 64 of outA (head A) and outB (head B)
            with nc.allow_low_precision(reason="fp16 rowsum"):
                nc.vector.tensor_copy(recip[64:65, 0:512], outA[64:65, :])
                nc.vector.tensor_copy(recip[64:65, 512:1024], outB[64:65, :])
            bc_ps = ps.tile([128, 1024], F32, name="bc", tag="sc", bufs=2)
            nc.tensor.matmul(bc_ps[0:64, 0:512], sel_t[:], recip[:, 0:512],
                             start=True, stop=True)
            nc.tensor.matmul(bc_ps[0:64, 512:1024], sel_t[:], recip[:, 512:1024],
                             start=True, stop=True)
            bc_sb = sbo.tile([64, 1024], F32, name="bc_sb", tag="bcastr", bufs=2)
            nc.vector.reciprocal_approx_fast(bc_sb[:], bc_ps[0:64, :])
            with nc.allow_low_precision(reason="fp16 out"):
                nc.vector.tensor_mul(outsc[pair][0:64, qb * 512:(qb + 1) * 512],
                                     outA[0:64, :], bc_sb[:, 0:512])
                bB = sbo.tile([64, 512], F16, name="bB", tag="bB", bufs=2)
                nc.vector.tensor_mul(bB[:], outB[0:64, :], bc_sb[:, 512:1024])
            # partition shift 0:64 -> 64:128 (engines cannot cross partitions)
            nc.gpsimd.dma_start(outsc[pair][64:128, qb * 512:(qb + 1) * 512],
                                bB[:])

        # ---- attention: flat cross-block software pipeline ----------------
        # blocks pair-interleaved so p3 lumps land next to slack; attn@V
        # trails scores/exp by one iteration across block boundaries; norm
        # of block b runs at iter 2 of block b+1; p3 of query block qb is
        # spread one ypb-piece per iteration over iters 8..15 of block
        # 2*qb+2 (outsc for qb complete by then, incl. the shift DMAs)
        blocks = [(pair, qb) for qb in range(NQB) for pair in range(2)]
        outs = {}
        ets = {}

        def attn_v(i):
            bi, kc = divmod(i, NKC)
            pair, qb = blocks[bi]
            outA, outB = outs[bi]
            et = ets.pop(i)
            vt = v_tiles[kc]
            vbase = pair * 130
            # [v|1] blocks: rows 0:64 = dims, row 64 = scaled rowsum
            nc.tensor.matmul(outA[0:65, :],
                             vt[:, vbase:vbase + 65],
                             et[:, 0:512],
                             start=(kc == 0), stop=(kc == NKC - 1))
            nc.tensor.matmul(outB[0:65, :],
                             vt[:, vbase + 65:vbase + 130],
                             et[:, 512:1024],
                             start=(kc == 0), stop=(kc == NKC - 1))

        ysb_cur = None
        for i in range(len(blocks) * NKC):
            bi, kc = divmod(i, NKC)
            pair, qb = blocks[bi]
            ktp, qtp = kt_t[pair], qt_t[pair]
            if kc == 0:
                outs[bi] = (
                    ps.tile([128, 512], F32, name="outA", tag="oA", bufs=2),
                    ps.tile([128, 512], F32, name="outB", tag="oB", bufs=2))
            sc = ps.tile([128, 1024], F32, name="sc", tag="sc", bufs=2)
            nc.tensor.matmul(sc[:, 0:512],
                             ktp[0:64, kc * 128:(kc + 1) * 128],
                             qtp[0:64, qb * 512:(qb + 1) * 512],
                             start=True, stop=True)
            nc.tensor.matmul(sc[:, 512:1024],
                             ktp[64:128, kc * 128:(kc + 1) * 128],
                             qtp[64:128, qb * 512:(qb + 1) * 512],
                             start=True, stop=True)
            et = sbe.tile([128, 1024], F16, name="et", tag="et", bufs=6)
            ets[i] = et
            with nc.allow_low_precision(reason="fp16 attn weights"):
                nc.scalar.activation(et[:], sc[:], Exp, scale=0.125)
            if kc == 2 and bi > 0:
                pb, qbb = blocks[bi - 1]
                norm(pb, qbb, *outs.pop(bi - 1))
            # p3(qX) spread: pieces 0-3 at block 2X+2 kc 8,10,12,14;
            # pieces 4-7 at block 2X+3 kc 0,2,4,6 (norm(1,qX) and its
            # shift DMA complete well before kc 8 of block 2X+2)
            piece = None
            if bi >= 2 and bi % 2 == 0 and kc >= 8 and kc % 2 == 0:
                piece = (blocks[bi - 2][1], (kc - 8) // 2)
            elif bi >= 3 and bi % 2 == 1 and kc <= 6 and kc % 2 == 0:
                piece = (blocks[bi - 3][1], 4 + kc // 2)
            if piece is not None:
                p3qb, ypb = piece
                if ypb == 0:
                    ysb_cur = sbo.tile([128, 8 * 512], F16, name="ysb",
                                       tag="ysb", bufs=2)
                p3_piece(p3qb, ypb, ysb_cur)
            if i > 0:
                attn_v(i - 1)
        attn_v(len(blocks) * NKC - 1)
        pb, qbb = blocks[-1]
        norm(pb, qbb, *outs.pop(len(blocks) - 1))
        p3(NQB - 1)

    nc.compile()
    return nc


def _get_nc():
    if "nc" not in _CACHE:
        _CACHE["nc"] = _build_nc()
    return _CACHE["nc"]


def kernel(q, k, v, Wq, bq, Wk, bk, Wv, bv, Wo, bo, _trace=False, _tmpdir=None):
    from concourse.bass_utils import run_bass_kernel_spmd

    q = np.asarray(q, np.float32)
    k = np.asarray(k, np.float32)
    v = np.asarray(v, np.float32)
    Wq = np.asarray(Wq, np.float32)
    Wk = np.asarray(Wk, np.float32)
    Wv = np.asarray(Wv, np.float32)
    Wo = np.asarray(Wo, np.float32)
    bq = np.asarray(bq, np.float32)
    bk = np.asarray(bk, np.float32)
    bv = np.asarray(bv, np.float32)
    bo = np.asarray(bo, np.float32)

    nc = _get_nc()

    # broadcast-selector for the row-sum reciprocal: bc rows 0:64 get
    # recip row 64 (rsA), rows 64:128 get recip row 63 (rsB fused) or
    # row 96 (rsB via the fallback M=1 matmul)
    sel = np.zeros((128, 64), np.float16)
    sel[64, :] = 1.0
    zr = np.zeros((128, 1024), np.float16)
    ones = np.ones((128, 1), np.float16)

    xT = {}
    for b in range(B):
        xT[("q", b)] = np.ascontiguousarray(q[b].T).astype(np.float16)
        xT[("k", b)] = np.ascontiguousarray(k[b].T).astype(np.float16)
        xT[("v", b)] = np.ascontiguousarray(v[b].T).astype(np.float16)

    # Effective K weights: bk drops out of softmax entirely (adds a
    # per-query-row constant to the scores).  Q bias applied on device.
    in_maps = []
    for c in range(8):
        b, g = c // G, c % G
        gr = slice(g * DH, (g + 1) * DH)
        in_maps.append({
            "xq": xT[("q", b)],
            "xk": xT[("k", b)],
            "xv": xT[("v", b)],
            "wq": np.ascontiguousarray(Wq[gr, :].T).astype(np.float16).reshape(KCD, 128, DH),
            "wk": np.ascontiguousarray(Wk[gr, :].T).astype(np.float16).reshape(KCD, 128, DH),
            "wv": np.ascontiguousarray(Wv[gr, :].T).astype(np.float16).reshape(KCD, 128, DH),
            "wo": np.ascontiguousarray(Wo[:, gr].T / 64.0).astype(np.float16).reshape(2, 128, D),
            "bq": np.ascontiguousarray(bq[gr].reshape(2, 128).T),
            "sel": sel,
            "zr": zr,
            "ones": ones,
        })

    kwargs = {}
    if _trace:
        kwargs = dict(trace=True, tmpdir=_tmpdir)
    res = run_bass_kernel_spmd(nc, in_maps, core_ids=list(range(8)), **kwargs)

    # host reduce: y[b] = sum_g y_g^T.T  (+ bias terms folded host-side)
    bias_row = bv @ Wo.T + bo                     # [D]
    out = np.empty((B, S, D), np.float32)
    for b in range(B):
        acc = np.zeros((S, D), np.float32)
        for g in range(G):
            acc += res.results[b * G + g]["y"].T.astype(np.float32)
        out[b] = acc + bias_row[None, :]
    if _trace:
        out = (out, res)
    return out
